# revision 4
# baseline (speedup 1.0000x reference)
"""Autoregressive LSTM decompressor on 8 Trainium2 NeuronCores, v3.

Replicated-recurrence design: the combined recurrent weight
W_sum = W_ih + W_hh (the reference feeds h back as the next input, so the
two per-step matmuls collapse for t>=1) is quantized to fp8 e3m4 at scale
64 (7.4e-3 end-to-end rel err vs fp32) which shrinks it to 16.8MB -- small
enough for EVERY core to keep the whole matrix SBUF-resident. Each core
then computes the full LSTM recurrence locally (identical, redundant) with
zero inter-core communication; only the output projection is sharded
(core r computes y rows [128r, 128r+128)).

Per step: one K=64 bias matmul + 1024 weight-stationary matmuls
(lhsT = fp8 [128,128] W tile, rhs = one bf16 h chunk [128,1]) accumulate
all 8192 gates in COLUMN layout in one PSUM tile [128, 64]
(cols = [i(16) | f(16) | o(16) | g(16)], unit = 128*(col%16) + partition).
The gate nonlinearities run on [128, 16..48] column tiles (ACT descales
the x64 weight/bias scale via activation's input scale), and h lands
directly in the local hist buffer -- no transposes, no collectives, no
RDMA, no semaphores. Step 0 (gates_0 = x @ W_ih.T + b, h0=c0=0) is folded
into the host-side input prep like the W_ih+W_hh/bias folds.
"""

import numpy as np
import ml_dtypes

D = 2048
DOUT = 1024
L = 256
NCORES = 8
KC = 16            # contraction chunks of 128
NJ = 64            # gate psum columns
SCALE = 32.0       # fp8 quantization scale for i,f,o rows (descaled in ACT)
# g rows carry 2*W at the same lattice (scale 64) so the single sigmoid pass
# yields sig(2g), and tanh(g) = 2*sig(2g) - 1 is a fused DVE op
_BF16 = ml_dtypes.bfloat16
_E3M4 = ml_dtypes.float8_e3m4

# psum gate-column blocks [i | f | o | g]; W row blocks are i,f,g,o
_GBASE = [0, 2048, 6144, 4096]


def _rows64():
    rows = np.empty((NJ, 128), np.int64)
    for j in range(NJ):
        rows[j] = _GBASE[j // 16] + 128 * (j % 16) + np.arange(128)
    return rows


def _prep_core_inputs(x, W_ih, W_sum, b, W_out):
    x = np.asarray(x, np.float32).reshape(-1)
    rows = _rows64()
    # [k, c, j, m] = Wq[rows[j, m], 128c + k]; g rows carry 2W (scale 64)
    rs = np.where(np.arange(NJ) >= 48, 2.0 * SCALE, SCALE)[:, None, None]
    Wq = (W_sum[rows] * rs).astype(_E3M4)
    blk = Wq.reshape(NJ, 128, KC, 128)
    wsum8 = np.ascontiguousarray(blk.transpose(3, 2, 0, 1))
    wb = (b[rows] * rs[:, :, 0]).astype(_BF16)        # [64, 128]
    ident = np.eye(NJ, dtype=np.float32).astype(_BF16)
    g0 = (W_ih @ x + b).astype(np.float32)            # host step-0 gates
    g0 = np.ascontiguousarray(g0[rows.T] * rs[:, 0, 0][None])  # [128, 64]
    in_maps = []
    for r in range(NCORES):
        wo_rows = 128 * r + np.arange(128)
        wo = W_out[wo_rows].reshape(128, KC, 128).transpose(2, 1, 0)
        in_maps.append({
            "wsum8": wsum8,
            "wb": wb,
            "ident": ident,
            "g0": g0,
            "wout": np.ascontiguousarray(wo).astype(_BF16),
        })
    return in_maps


def _build_program(nsteps=L):
    from concourse import bacc, tile, mybir

    dt = mybir.dt
    nc = bacc.Bacc("TRN2", target_bir_lowering=False, debug=False,
                   num_devices=NCORES)

    wsum_d = nc.dram_tensor("wsum8", [128, KC, NJ, 128], dt.float8e3,
                            kind="ExternalInput")
    wb_d = nc.dram_tensor("wb", [NJ, 128], dt.bfloat16, kind="ExternalInput")
    ident_d = nc.dram_tensor("ident", [NJ, NJ], dt.bfloat16,
                             kind="ExternalInput")
    g0_d = nc.dram_tensor("g0", [128, NJ], dt.float32, kind="ExternalInput")
    wout_d = nc.dram_tensor("wout", [128, KC, 128], dt.bfloat16,
                            kind="ExternalInput")
    y_d = nc.dram_tensor("y", [128, nsteps], dt.float32,
                         kind="ExternalOutput")

    Sig = mybir.ActivationFunctionType.Sigmoid
    Tanh = mybir.ActivationFunctionType.Tanh
    DS = 1.0 / SCALE

    with tile.TileContext(nc) as tc:
        with (
            tc.tile_pool(name="wpool", bufs=1) as wpool,
            tc.tile_pool(name="state", bufs=1) as state,
            tc.tile_pool(name="work", bufs=3) as work,
            tc.tile_pool(name="psum", bufs=2, space="PSUM") as psum,
        ):
            wsum8 = wpool.tile([128, KC, NJ, 128], dt.float8e3)
            wb = wpool.tile([NJ, 128], dt.bfloat16)
            ident = wpool.tile([NJ, NJ], dt.bfloat16)
            g0 = wpool.tile([128, NJ], dt.float32)
            wout = wpool.tile([128, KC, 128], dt.bfloat16)
            hist = state.tile([128, nsteps, KC], dt.bfloat16)
            tcat = state.tile([128, 32], dt.float32)  # [tanh_g | c]
            ones16 = state.tile([128, 16], dt.float32)

            nc.sync.dma_start(wb[:], wb_d[:])
            nc.sync.dma_start(ident[:], ident_d[:])
            nc.sync.dma_start(g0[:], g0_d[:])
            # split the 16.8MB weight load across all DMA-capable engines
            nc.sync.dma_start(wsum8[:, 0:6], wsum_d[:, 0:6])
            nc.scalar.dma_start(wsum8[:, 6:12], wsum_d[:, 6:12])
            nc.gpsimd.dma_start(wsum8[:, 12:16], wsum_d[:, 12:16])
            nc.gpsimd.dma_start(wout[:], wout_d[:])
            nc.vector.memset(tcat[:], 0.0)
            nc.vector.memset(ones16[:], 1.0)
            yp = psum.tile([128, nsteps], dt.float32, tag="yp")

            for t in range(nsteps):
                if t > 0:
                    g = psum.tile([128, NJ], dt.float32, tag="g")
                    nc.tensor.matmul(g[:], wb[:], ident[:],
                                     start=True, stop=False)
                    for c in range(KC):
                        rhs = hist[:, t - 1, c:c + 1]
                        for j in range(NJ):
                            nc.tensor.matmul(
                                g[:, j:j + 1], wsum8[:, c, j, :], rhs,
                                start=False,
                                stop=(c == KC - 1 and j == NJ - 1))
                    gsrc = g
                else:
                    gsrc = g0
                s_ifo = work.tile([128, 64], dt.float32, tag="sifo")
                m12 = work.tile([128, 32], dt.float32, tag="m12")
                tct = work.tile([128, 16], dt.float32, tag="tct")
                nc.scalar.activation(s_ifo[:], gsrc[:], Sig, scale=DS)
                nc.vector.scalar_tensor_tensor(
                    tcat[:, 0:16], s_ifo[:, 48:64], 2.0, ones16[:],
                    op0=mybir.AluOpType.mult, op1=mybir.AluOpType.subtract)
                nc.vector.tensor_mul(m12[:], s_ifo[:, 0:32], tcat[:])
                nc.vector.tensor_add(tcat[:, 16:32], m12[:, 0:16],
                                     m12[:, 16:32])
                nc.scalar.activation(tct[:], tcat[:, 16:32], Tanh)
                nc.vector.tensor_mul(hist[:, t, :], s_ifo[:, 32:48], tct[:])
                # projection column t: 16 free PE matmuls, overlapped
                for c in range(KC):
                    nc.tensor.matmul(yp[:, t:t + 1], wout[:, c, :],
                                     hist[:, t, c:c + 1],
                                     start=(c == 0), stop=(c == KC - 1))

            ysb = work.tile([128, nsteps], dt.float32, tag="ysb")
            nc.vector.tensor_copy(ysb[:], yp[:])
            nc.sync.dma_start(y_d[:], ysb[:])

    nc.compile()
    return nc


def kernel(x, W_ih, W_hh, b_ih, b_hh, W_out, b_out, seq_len, _trace=False):
    from concourse.bass_utils import run_bass_kernel_spmd

    assert int(seq_len) == L
    x = np.asarray(x, np.float32)
    W_ih = np.asarray(W_ih, np.float32)
    b = np.asarray(b_ih, np.float32) + np.asarray(b_hh, np.float32)
    W_sum = W_ih + np.asarray(W_hh, np.float32)
    W_out = np.asarray(W_out, np.float32)
    b_out = np.asarray(b_out, np.float32)

    in_maps = _prep_core_inputs(x, W_ih, W_sum, b, W_out)
    nc = _build_program()
    res = run_bass_kernel_spmd(nc, in_maps, list(range(NCORES)),
                               trace=_trace)
    yfull = np.empty((DOUT, L), np.float32)
    for r in range(NCORES):
        yfull[128 * r:128 * (r + 1)] = np.asarray(res.results[r]["y"],
                                                  np.float32)
    out = (yfull.T + b_out)[None]  # [1, L, DOUT]
    if _trace:
        return out, res
    return out
